# revision 13
# baseline (speedup 1.0000x reference)
"""Trainium2 Bass kernel for nn_BinCat (embedding_lookup).

Reference computation:
    idx[n] = sum_j (1 - x[n, j]) * 2^(L-1-j)      # x bits in {0,1}, L=20
    out[n] = cats[idx[n]]                          # cats: [2^20, 64] f32

Strategy (8 NeuronCores, data parallel):
  - Replicate cats (256 MB) to every core; shard x rows across cores.
  - Per core (N=8192 rows): load x as [128, T=64, L=20] int32, compute
    idx = reduce_add((x ^ 1) << (L-1-j)) on the vector engine, then gather
    with gpsimd indirect DMAs — the HW consumes exactly one index per
    destination partition per instruction, so a [128, TC] index tile takes
    TC instructions, each fetching 128 random 256 B rows — and store each
    chunk with one wide contiguous HWDGE DMA.
  - Measured on HW: ~101 us/core/iteration, ~1.5 us per indirect DMA of
    Pool-engine (Q7 SWDGE descriptor-gen) time, which is the bottleneck;
    transfers (4.6 MB/core) hide underneath it entirely.
"""

import numpy as np

import concourse.bass as bass
import concourse.bacc as bacc
import concourse.mybir as mybir
import concourse.tile as tile
from concourse.bass_utils import run_bass_kernel_spmd

P = 128          # SBUF partitions
L = 20           # bits per index
D = 64           # embedding dim
BATCH = 4096
I = 16
NCORES = 8
N = BATCH * I // NCORES   # rows per core = 8192
T = N // P                # rows per partition = 64

# Pipeline chunking along T (each chunk = P*TC rows). 1 = single shot.
N_CHUNKS = 4
TC = T // N_CHUNKS

_CACHE: dict[str, object] = {}


def build_bass(reps: int = 1):
    nc = bacc.Bacc("TRN2", target_bir_lowering=False, debug=False)

    x = nc.dram_tensor("x", [N, L], mybir.dt.int32, kind="ExternalInput")
    cats = nc.dram_tensor("cats", [2 ** L, D], mybir.dt.float32, kind="ExternalInput")
    out = nc.dram_tensor("out", [N, D], mybir.dt.float32, kind="ExternalOutput")

    # Row n = p*T + t lives in partition p, free slot t: per-partition
    # blocks of T rows stay contiguous in DRAM for both load and store.
    x_v = x.ap().rearrange("(p t) j -> p (t j)", p=P)
    out_v = out.ap().rearrange("(p t) d -> p (t d)", p=P)

    import contextlib

    with tile.TileContext(nc) as tc:
        with (
            tc.tile_pool(name="sbuf", bufs=2) as pool,
            tc.tile_pool(name="const", bufs=1) as cpool,
        ):
            # Shift amounts (L-1-j) replicated across the chunk's free dim.
            amt = cpool.tile([P, TC * L], mybir.dt.int32, tag="amt")
            nc.gpsimd.iota(
                amt[:], pattern=[[0, TC], [-1, L]], base=L - 1, channel_multiplier=0
            )

            # reps>1 wraps the body in a HW loop — benchmarking only (the
            # axon RPC round-trip is ~80 ms, so per-call wall time needs the
            # body repeated enough to dominate).
            loop = tc.For_i(0, reps, 1) if reps > 1 else contextlib.nullcontext()
            with loop:
                for c in range(N_CHUNKS):
                    xs = slice(c * TC * L, (c + 1) * TC * L)
                    x_t = pool.tile([P, TC * L], mybir.dt.int32, tag="x")
                    nc.sync.dma_start(out=x_t[:], in_=x_v[:, xs])

                    # y = (x ^ 1) << amt  ==  (1 - x_j) * 2^(L-1-j)
                    y = pool.tile([P, TC * L], mybir.dt.int32, tag="y")
                    nc.vector.tensor_scalar(
                        out=y[:],
                        in0=x_t[:],
                        scalar1=1,
                        scalar2=None,
                        op0=mybir.AluOpType.bitwise_xor,
                    )
                    nc.vector.tensor_tensor(
                        out=y[:],
                        in0=y[:],
                        in1=amt[:],
                        op=mybir.AluOpType.logical_shift_left,
                    )

                    idx = pool.tile([P, TC], mybir.dt.int32, tag="idx")
                    with nc.allow_low_precision(reason="int32 bit-sum is exact"):
                        nc.vector.tensor_reduce(
                            out=idx[:],
                            in_=y[:].rearrange("p (t j) -> p t j", j=L),
                            axis=mybir.AxisListType.X,
                            op=mybir.AluOpType.add,
                        )

                    # HW contract: one index per partition per indirect DMA,
                    # each gathering one contiguous D-row into its partition.
                    g = pool.tile([P, TC * D], mybir.dt.float32, tag="g")
                    for t in range(TC):
                        nc.gpsimd.indirect_dma_start(
                            out=g[:, t * D : (t + 1) * D],
                            out_offset=None,
                            in_=cats.ap(),
                            in_offset=bass.IndirectOffsetOnAxis(
                                ap=idx[:, t : t + 1], axis=0
                            ),
                        )

                    os = slice(c * TC * D, (c + 1) * TC * D)
                    nc.sync.dma_start(out=out_v[:, os], in_=g[:])

    nc.compile()
    return nc


def _get_nc():
    if "nc" not in _CACHE:
        _CACHE["nc"] = build_bass()
    return _CACHE["nc"]


def kernel(x: np.ndarray, cats: np.ndarray) -> np.ndarray:
    x = np.asarray(x)
    cats = np.ascontiguousarray(np.asarray(cats, dtype=np.float32))
    assert x.shape == (BATCH, I, L) and x.dtype == np.int32
    assert cats.shape == (2 ** L, D)

    nc = _get_nc()
    x_flat = np.ascontiguousarray(x.reshape(BATCH * I, L))
    in_maps = [
        {"x": x_flat[i * N : (i + 1) * N], "cats": cats} for i in range(NCORES)
    ]
    res = run_bass_kernel_spmd(nc, in_maps, core_ids=list(range(NCORES)))
    out = np.concatenate([r["out"] for r in res.results], axis=0)
    return out.reshape(BATCH, I, D)
